# revision 22
# baseline (speedup 1.0000x reference)
"""Self-attention kernel for TRN2, data-parallel over batch (8 cores), fp8.

Per core (one batch element x[2048, 512]):
  - x^T is prepared on the HOST in fp8 ([P, NTB, CCH, TBLK] layout), so no
    on-chip transposes are needed; the residual x (+ folded bias bc) is
    shipped bf16 and the output is written bf16 (host casts back to fp32).
  - q/k/v projections and all attention matmuls run fp8 with
    perf_mode=DoubleRow (contraction pairs of 128-chunks -> ~2x TensorE).
  - scores computed TRANSPOSED (sT[s,t]) so the exp output feeds PV
    directly; exp = e^{score/16 - 2} (bias cancels in normalization),
    fused over two PSUM banks per activation.
  - PV runs lag-1 behind the exp; row sums come free via a ones-column
    in v; the reciprocal is folded into the bf16 cast of a.
  - block 0's score streak absorbs the k/v projection chunks the same way
    later blocks absorb the previous block's transposes/output projection;
    two output-projection chunks are held back into each sweep B so the
    PE has work while ScalarE drains the block-boundary exp backlog.
  - DMA issue cost (~0.6us per dma_start on a sequencer) is split across
    three queues: inputs on Sync + Scalar, outputs on GpSimd.
  - biases: bq/bk exact via per-partition add; bv/ba folded on the HOST
    into bc = Wa^T bv + ba, added into the bf16 residual (exact:
    attention rows sum to 1).

Matmul inputs fp8e4, PSUM accumulation fp32, softmax/normalize fp32,
residual + output bf16.
"""

import ml_dtypes
import numpy as np

import concourse.bass as bass
import concourse.mybir as mybir
import concourse.tile as tile
from concourse import bacc
from concourse.bass import ds, ts
from concourse.bass_utils import run_bass_kernel_spmd
from concourse.masks import make_identity

F32 = mybir.dt.float32
BF16 = mybir.dt.bfloat16
F8 = mybir.dt.float8e4
AF = mybir.ActivationFunctionType
DR = mybir.MatmulPerfMode.DoubleRow

B, T, C, U, P = 8, 2048, 512, 256, 128
TC = T // P    # 16 row tiles
CCH = C // P   # 4 c-chunks
UCH = U // P   # 2 u-chunks
TBLK = 512     # t-block for attention
NTB = T // TBLK  # 4
VF = U + 16    # v free dim padded so the pair-dim stride is 16B-aligned
SCALE = 1.0 / float(np.sqrt(U))
EXPB = -2.0    # exp bias; cancels in row-sum normalization

_cache = {}


WBYT = 3 * CCH * U + 16   # W3 | bqk bytes


def _build_kernel(tc):
    nc = tc.nc
    # one dma_start moves ~100-160GB/s and transfers are FIFO per issuing
    # ring, so inputs are spread over the three rings (sync, scalar,
    # gpsimd-SWDGE) sized/ordered by consumption deadline
    Wblob = nc.dram_tensor("Wblob", [P, WBYT], F8, kind="ExternalInput").ap()
    xT8_0 = nc.dram_tensor("xT8_0", [P, CCH, TBLK], F8,
                           kind="ExternalInput").ap()
    xT8_1 = nc.dram_tensor("xT8_1", [P, CCH, TBLK], F8,
                           kind="ExternalInput").ap()
    xT8_23 = nc.dram_tensor("xT8_23", [P, 2, CCH, TBLK], F8,
                            kind="ExternalInput").ap()
    Wa8 = nc.dram_tensor("Wa8", [P, UCH, C], F8, kind="ExternalInput").ap()
    xpb = nc.dram_tensor("xpb", [P, TC, C], BF16, kind="ExternalInput").ap()
    out = nc.dram_tensor("out", [P, TC, C], BF16, kind="ExternalOutput").ap()

    consts = tc.alloc_tile_pool(name="consts", bufs=1)
    persist = tc.alloc_tile_pool(name="persist", bufs=1)

    identity = consts.tile([P, P], BF16)
    make_identity(nc, identity)

    # warm the ACT exp table early (one-time ~2.7us table load)
    dex = consts.tile([P, 1], F32)
    nc.vector.memset(dex, 0.0)
    expb = consts.tile([P, 1], F32)
    nc.vector.memset(expb, EXPB)
    dex2 = consts.tile([P, 1], F32)
    nc.scalar.activation(out=dex2, in_=dex, func=AF.Exp, bias=dex[:, 0:1],
                         scale=1.0)

    # persistent tensors
    Wblob_sb = persist.tile([P, WBYT], F8)
    xT8_sb = persist.tile([P, NTB, CCH, TBLK], F8)  # x^T (c on partitions)
    Wa_f8 = persist.tile([P, UCH, C], F8)
    xpb_sb = persist.tile([P, TC, C], BF16)   # residual x + bc, bf16
    qT_f8 = persist.tile([P, UCH, T], F8)     # q^T  (u on partitions)
    kT_f8 = persist.tile([P, UCH, T], F8)     # k^T
    v_sb = persist.tile([P, TC, VF], F8)      # v rows + ones col + pad
    aT_f8 = persist.tile([P, UCH, T], F8)     # a^T (normalized)
    nc.vector.memset(v_sb[:, :, U:VF], 0.0)
    nc.vector.memset(v_sb[:, :, U:U + 1], 1.0)

    # views into the weight blob
    W3_sb = Wblob_sb[:, 0:3 * CCH * U].rearrange(
        "p (w c u) -> p w c u", w=3, c=CCH)
    Wq_f8, Wk_f8, Wv_f8 = W3_sb[:, 0], W3_sb[:, 1], W3_sb[:, 2]
    bqk_sb = Wblob_sb[:, 3 * CCH * U:WBYT].bitcast(F32)  # [P, 4]
    bq_sb, bk_sb = bqk_sb[:, 0:2], bqk_sb[:, 2:4]

    def xT8v(tb):
        return xT8_sb[:, tb]

    # sync ring: weights+biases, then the later x^T slices
    nc.sync.dma_start(out=Wblob_sb, in_=Wblob)
    nc.sync.dma_start(out=xT8_sb[:, 1], in_=xT8_1)
    nc.sync.dma_start(out=xT8_sb[:, 2:4], in_=xT8_23)
    # scalar ring: first x^T slice (preamble-critical), output weights
    nc.scalar.dma_start(out=xT8_sb[:, 0], in_=xT8_0)
    nc.scalar.dma_start(out=Wa_f8, in_=Wa8)
    # gpsimd (SWDGE) ring: the residual, needed only from block 1 on
    nc.gpsimd.dma_start(out=xpb_sb[:, 0:TC // 2, :], in_=xpb[:, 0:TC // 2, :])
    nc.gpsimd.dma_start(out=xpb_sb[:, TC // 2:, :], in_=xpb[:, TC // 2:, :])

    warm_rhs = consts.tile([P, TBLK], BF16)
    nc.vector.memset(warm_rhs, 0.0)

    # PSUM pools for the whole kernel: 4 (scores) + 2 (acc) + 2 (misc)
    spsum = tc.alloc_tile_pool(name="spsum", bufs=2, space="PSUM")
    apsum = tc.alloc_tile_pool(name="apsum", bufs=2, space="PSUM")
    p_pool = tc.alloc_tile_pool(name="p_pool", bufs=14)
    abf_pool = tc.alloc_tile_pool(name="abf_pool", bufs=8)
    rcp_pool = tc.alloc_tile_pool(name="rcp_pool", bufs=3)
    y_pool = tc.alloc_tile_pool(name="y_pool", bufs=2)

    # PE warmup during the DMA stream (no data deps): bridges the wait
    # for the first input DMA and ramps the HAM power state (~4us of
    # continuous PE activity needed for full clock)
    wtile = apsum.tile([P, TBLK], F32, tag="misc", name="warmup")
    for i in range(8):
        nc.tensor.matmul(wtile, lhsT=identity, rhs=warm_rhs,
                         start=(i == 0), stop=(i == 7))

    def proj_chunk(W_f8, bias_sb, dst, uc, tb, eng_act):
        """q/k projection for one u-chunk x one t-block slice."""
        wps = apsum.tile([P, TBLK], F32, tag="misc", name="wps")
        for i in range(2):
            nc.tensor.matmul(
                wps,
                lhsT=W_f8[:, 2 * i:2 * i + 2, ts(uc, P)],
                rhs=xT8v(tb)[:, 2 * i:2 * i + 2, :],
                start=(i == 0), stop=(i == 1), perf_mode=DR,
            )
        if eng_act:
            nc.scalar.activation(
                out=dst[:, uc, ds(tb * TBLK, TBLK)], in_=wps,
                func=AF.Identity, bias=bias_sb[:, uc:uc + 1], scale=1.0,
            )
        else:
            nc.vector.tensor_scalar(
                out=dst[:, uc, ds(tb * TBLK, TBLK)], in0=wps,
                scalar1=bias_sb[:, uc:uc + 1], scalar2=None,
                op0=mybir.AluOpType.add,
            )

    def v_chunk(j, eng_act):
        """v projection for row tiles 2j, 2j+1 (one PSUM bank)."""
        tb, j0 = (2 * j) // 4, (2 * j) % 4
        vps = apsum.tile([P, 2, U], F32, tag="misc", name="vps")
        for jj in range(2):
            for i in range(2):
                nc.tensor.matmul(
                    vps[:, jj, :],
                    lhsT=xT8v(tb)[:, 2 * i:2 * i + 2,
                                  ds((j0 + jj) * P, P)],
                    rhs=Wv_f8[:, 2 * i:2 * i + 2, :],
                    start=(jj == 0 and i == 0),
                    stop=(jj == 1 and i == 1), perf_mode=DR,
                )
        t0 = 2 * j
        if eng_act:
            nc.scalar.copy(out=v_sb[:, t0:t0 + 2, 0:U], in_=vps)
        else:
            nc.vector.tensor_copy(out=v_sb[:, t0:t0 + 2, 0:U], in_=vps)

    # preamble: q^T block 0, k^T slice 0, v tiles 0-3 (needs xT8 slice 0).
    # Scalar/Vector alternate here (exp isn't running yet).
    for uc in range(UCH):
        proj_chunk(Wq_f8, bq_sb, qT_f8, uc, 0, uc == 0)
    for uc in range(UCH):
        proj_chunk(Wk_f8, bk_sb, kT_f8, uc, 0, True)
    v_chunk(0, True)
    v_chunk(1, False)

    def norm_cast(apss, abfs, tsl, eng_act=False):
        """rcp of row sum, then a_bf = aps * rcp (normalized), fp32->bf16."""
        aps = apss[tsl]
        rcp = rcp_pool.tile([P, 1], F32, tag="rcp")
        nc.vector.reciprocal(rcp, aps[:, U:U + 1])
        a_bf = abf_pool.tile([P, U], BF16, tag="abf")
        if eng_act:
            nc.scalar.activation(out=a_bf, in_=aps[:, 0:U], func=AF.Copy,
                                 scale=rcp[:, 0:1])
        else:
            nc.vector.tensor_scalar(
                out=a_bf, in0=aps[:, 0:U], scalar1=rcp, scalar2=None,
                op0=mybir.AluOpType.mult,
            )
        abfs[tsl] = a_bf

    def deferred_work(tb, abfs, single_dma=False):
        """Transposes of a (tb), then output projection + residual (tb).
        Returns (tchunks, fchunks); y tiles pair up for one DMA per two
        row tiles unless single_dma (used to shorten the final tail)."""
        tchunks, fchunks = [], []
        y2box = [None]
        for tsl in range(NTB):
            def tchunk(tsl=tsl, tb=tb, abfs=abfs):
                row0 = tb * TBLK + tsl * P
                atps = apsum.tile([P, UCH, P], F32, tag="misc", name="atps")
                for uc in range(UCH):
                    nc.tensor.matmul(
                        atps[:, uc, :], lhsT=abfs[tsl][:, ts(uc, P)],
                        rhs=identity,
                        start=(uc == 0), stop=(uc == UCH - 1),
                    )
                nc.vector.tensor_copy(out=aT_f8[:, :, ds(row0, P)], in_=atps)
            tchunks.append(tchunk)
        for tsl in range(NTB):
            def fchunk(tsl=tsl, tb=tb, y2box=y2box):
                row0 = tb * TBLK + tsl * P
                yps = apsum.tile([P, TBLK], F32, tag="misc", name="yps")
                nc.tensor.matmul(
                    yps, lhsT=aT_f8[:, :, ds(row0, P)], rhs=Wa_f8[:, :, :],
                    start=True, stop=True, perf_mode=DR,
                )
                if single_dma:
                    y1 = y_pool.tile([P, C], BF16, tag="ysb", name="y1")
                    nc.vector.tensor_add(
                        out=y1, in0=yps, in1=xpb_sb[:, tb * NTB + tsl, :]
                    )
                    # spread the final DMAs over rings that idle at the
                    # end so issue+transfer of the last tiles overlap
                    eng = (nc.gpsimd, nc.gpsimd, nc.sync, nc.scalar)[tsl]
                    eng.dma_start(out=out[:, tb * NTB + tsl, :], in_=y1)
                    return
                if tsl % 2 == 0:
                    y2box[0] = y_pool.tile([P, 2, C], BF16, tag="ysb",
                                           name="y2")
                y2 = y2box[0]
                nc.vector.tensor_add(
                    out=y2[:, tsl % 2, :], in0=yps,
                    in1=xpb_sb[:, tb * NTB + tsl, :]
                )
                if tsl % 2 == 1:
                    t0 = tb * NTB + tsl - 1
                    nc.gpsimd.dma_start(out=out[:, t0:t0 + 2, :], in_=y2)
            fchunks.append(fchunk)
        return tchunks, fchunks

    def emit_scp(tb, scp, pts):
        sps = spsum.tile([P, 2, TBLK], F32, tag="sps", name="sps")
        for j in range(2):
            nc.tensor.matmul(
                sps[:, j, :],
                lhsT=kT_f8[:, :, ts(2 * scp + j, P)],
                rhs=qT_f8[:, :, ds(tb * TBLK, TBLK)],
                start=True, stop=True, perf_mode=DR,
            )
        pt = p_pool.tile([P, 2, TBLK], F8, tag="pt", name="pt")
        nc.scalar.activation(out=pt, in_=sps, func=AF.Exp,
                             bias=expb[:, 0:1], scale=SCALE)
        pts.append(pt)

    # block 0's todo: remaining k^T slices / v tiles / q^T block 1, in an
    # order where each k^T slice lands before the scp that consumes it.
    # All on VectorE: ScalarE is saturated with exp during the streak.
    todo0 = []
    for tb in range(1, NTB):
        for uc in range(UCH):
            def kchunk(uc=uc, tb=tb):
                proj_chunk(Wk_f8, bk_sb, kT_f8, uc, tb, False)
            todo0.append(kchunk)
        for h in range(2):
            def vchunk(h=h, tb=tb):
                v_chunk(2 * tb + h, False)
            todo0.append(vchunk)

    deferred = []  # (streak chunks, sweep-B held-back chunks)
    heldB = []
    nextpts = []
    for tb in range(NTB):
        pts = nextpts  # scp0/1 may have been hoisted into tb-1's tail
        nextpts = []
        abfs = [None] * NTB
        apss = [None] * NTB
        for tsl in (0, 1):
            apss[tsl] = apsum.tile([P, VF], F32, tag="acc", name="apsA")
        todo = todo0 if tb == 0 else list(deferred)
        btodo = list(heldB)
        # qT for block tb+1 joins the streak queue (must finish before the
        # hoisted score groups of block tb+1 are emitted)
        if tb + 1 < NTB:
            for uc in range(UCH):
                def qchunk(uc=uc, tb=tb):
                    proj_chunk(Wq_f8, bq_sb, qT_f8, uc, tb + 1, False)
                todo.append(qchunk)
        npv = [0]  # sweep-A pairs emitted so far

        def pva_upto(limit):
            while npv[0] < limit:
                j = npv[0]
                for tsl in (0, 1):
                    nc.tensor.matmul(
                        apss[tsl],
                        lhsT=pts[j][:, :, ts(tsl, P)],
                        rhs=v_sb[:, 2 * j:2 * j + 2, :],
                        start=(j == 0), stop=False, perf_mode=DR,
                    )
                npv[0] += 1

        pop_at = 1 if tb == 0 else 2
        for scp in range(len(pts), 8):
            emit_scp(tb, scp, pts)
            # PV sweep A (row tiles 0,1), one pair behind the exp
            pva_upto(scp)
            # interleave prep/deferred chunks, at most two per score group
            # to keep the PE filler evenly spread through the streak
            if scp >= pop_at:
                for _ in range(2):
                    if todo:
                        todo.pop(0)()
        pva_upto(7)
        for tsl in (0, 1):
            nc.tensor.matmul(
                apss[tsl], lhsT=pts[7][:, :, ts(tsl, P)],
                rhs=v_sb[:, 14:16, :], start=False, stop=True, perf_mode=DR,
            )
        while todo:
            todo.pop(0)()
        if tb + 1 < NTB:
            # hoist the next block's first two score groups so their
            # exps run on ScalarE while sweep B occupies the PE
            emit_scp(tb + 1, 0, nextpts)
            emit_scp(tb + 1, 1, nextpts)
        norm_cast(apss, abfs, 0)
        norm_cast(apss, abfs, 1)
        # PV sweep B (row tiles 2,3) over the retained p tiles, padded
        # with held-back chunks so the PE stays busy while ScalarE drains
        # the boundary exp backlog: the previous block's last two output
        # projections plus THIS block's first two a-transposes (they only
        # need norms 0,1, done just above)
        last = tb == NTB - 1
        for tsl in (2, 3):
            apss[tsl] = apsum.tile([P, VF], F32, tag="acc", name="apsB")
        if not last:
            tch, fch = deferred_work(tb, abfs)
            btodo = btodo + [tch[0], tch[1]]
            for scp in range(8):
                for tsl in (2, 3):
                    nc.tensor.matmul(
                        apss[tsl],
                        lhsT=pts[scp][:, :, ts(tsl, P)],
                        rhs=v_sb[:, 2 * scp:2 * scp + 2, :],
                        start=(scp == 0), stop=(scp == 7), perf_mode=DR,
                    )
                if scp % 2 == 1 and btodo:
                    btodo.pop(0)()
            norm_cast(apss, abfs, 2, True)
            norm_cast(apss, abfs, 3, True)
            while btodo:
                btodo.pop(0)()
            deferred = [fch[0], fch[1], tch[2], tch[3]]
            heldB = [fch[2], fch[3]]
        else:
            # last block: tile-SERIAL sweep B so tile 2's norm/transpose/
            # output-proj/DMA chain overlaps tile 3's PV, and only tile
            # 3's short chain remains as the tail (single-tile DMAs)
            tch, fch = deferred_work(tb, abfs, single_dma=True)
            btodo = btodo + [tch[0], tch[1], fch[0], fch[1]]
            for tsl in (2, 3):
                for scp in range(8):
                    nc.tensor.matmul(
                        apss[tsl],
                        lhsT=pts[scp][:, :, ts(tsl, P)],
                        rhs=v_sb[:, 2 * scp:2 * scp + 2, :],
                        start=(scp == 0), stop=(scp == 7), perf_mode=DR,
                    )
                    if scp % 2 == 1 and btodo:
                        btodo.pop(0)()
                if tsl == 2:
                    norm_cast(apss, abfs, 2, True)
                    btodo = btodo + [tch[2], fch[2]]
            norm_cast(apss, abfs, 3, True)
            while btodo:
                btodo.pop(0)()
            tch[3]()
            fch[3]()

    for pool in (y_pool, rcp_pool, abf_pool, p_pool,
                 apsum, spsum, persist, consts):
        pool.release()


def _get_nc():
    if "nc" not in _cache:
        nc = bacc.Bacc("TRN2", target_bir_lowering=False, debug=False)
        with tile.TileContext(nc) as tc:
            _build_kernel(tc)
        nc.compile()
        _cache["nc"] = nc
    return _cache["nc"]


def _w8(w, chunks):
    """fp32 [K, N] -> fp8e4m3 [P, K//P, N] with K-chunk layout for lhsT."""
    f8 = w.reshape(chunks, P, -1).transpose(1, 0, 2)
    return np.ascontiguousarray(f8.astype(ml_dtypes.float8_e4m3))


def _host_inputs(inputs):
    f32 = np.float32
    f8 = ml_dtypes.float8_e4m3
    Wa = np.ascontiguousarray(np.asarray(inputs["Wa"], dtype=f32))
    bc = np.asarray(inputs["bv"], dtype=f32) @ Wa + np.asarray(
        inputs["ba"], dtype=f32
    )
    W3 = np.stack([
        _w8(np.asarray(inputs["Wq"], dtype=f32), CCH),
        _w8(np.asarray(inputs["Wk"], dtype=f32), CCH),
        _w8(np.asarray(inputs["Wv"], dtype=f32), CCH),
    ], axis=1).reshape(P, -1)  # [P, 3*CCH*U]
    bqk = np.stack([
        np.asarray(inputs["bq"], dtype=f32).reshape(UCH, P).T,
        np.asarray(inputs["bk"], dtype=f32).reshape(UCH, P).T,
    ], axis=1)  # [P, 2, UCH]
    bqk_bytes = np.ascontiguousarray(bqk).view(np.uint8).reshape(P, 16)
    Wa_b = _w8(Wa, UCH).reshape(P, -1)
    Wblob = np.ascontiguousarray(
        np.concatenate([W3, bqk_bytes.view(f8)], axis=1))
    Wa8 = np.ascontiguousarray(Wa_b.reshape(P, UCH, C))
    xs = np.asarray(inputs["x"], dtype=f32)
    maps = []
    for b in range(B):
        # xT8[p, tb, cc, t'] = x[b][tb*TBLK + t', cc*P + p]  in fp8
        xt = xs[b].T.astype(f8)  # [C, T]
        xt = xt.reshape(CCH, P, NTB, TBLK).transpose(1, 2, 0, 3)
        # xpb[p, tt, c] = x[tt*P + p, c] + bc[c]  in bf16
        xpb = (xs[b] + bc).astype(ml_dtypes.bfloat16)
        xpb = xpb.reshape(TC, P, C).transpose(1, 0, 2)
        maps.append({
            "Wblob": Wblob,
            "Wa8": Wa8,
            "xT8_0": np.ascontiguousarray(xt[:, 0]),
            "xT8_1": np.ascontiguousarray(xt[:, 1]),
            "xT8_23": np.ascontiguousarray(xt[:, 2:4]),
            "xpb": np.ascontiguousarray(xpb),
        })
    return maps


def _unshard_out(o):
    """[P, TC, C] bf16 -> [T, C] f32."""
    return np.asarray(o).transpose(1, 0, 2).reshape(T, C).astype(np.float32)


def kernel(**inputs):
    nc = _get_nc()
    in_maps = _host_inputs(inputs)
    res = run_bass_kernel_spmd(nc, in_maps, core_ids=list(range(B)))
    return np.stack(
        [_unshard_out(res.results[b]["out"]) for b in range(B)], axis=0
    )


# revision 26
# speedup vs baseline: 1.0028x; 1.0028x over previous
"""Self-attention kernel for TRN2, data-parallel over batch (8 cores), fp8.

Per core (one batch element x[2048, 512]):
  - x^T is prepared on the HOST in fp8 ([P, NTB, CCH, TBLK] layout), so no
    on-chip transposes are needed; the residual x (+ folded bias bc) is
    shipped bf16 and the output is written bf16 (host casts back to fp32).
  - q/k/v projections and all attention matmuls run fp8 with
    perf_mode=DoubleRow (contraction pairs of 128-chunks -> ~2x TensorE).
  - scores computed TRANSPOSED (sT[s,t]) so the exp output feeds PV
    directly; exp = e^{score/16 - 2} (bias cancels in normalization),
    fused over two PSUM banks per activation.
  - PV runs lag-1 behind the exp; row sums come free via a ones-column
    in v; the reciprocal is folded into the bf16 cast of a.
  - block 0's score streak absorbs the k/v projection chunks the same way
    later blocks absorb the previous block's transposes/output projection;
    two output-projection chunks are held back into each sweep B so the
    PE has work while ScalarE drains the block-boundary exp backlog.
  - DMA issue cost (~0.6us per dma_start on a sequencer) is split across
    three queues: inputs on Sync + Scalar, outputs on GpSimd.
  - biases: bq/bk exact via per-partition add; bv/ba folded on the HOST
    into bc = Wa^T bv + ba, added into the bf16 residual (exact:
    attention rows sum to 1).

Matmul inputs fp8e4, PSUM accumulation fp32, softmax/normalize fp32,
residual + output bf16.
"""

import ml_dtypes
import numpy as np

import concourse.bass as bass
import concourse.mybir as mybir
import concourse.tile as tile
from concourse import bacc
from concourse.bass import ds, ts
from concourse.bass_utils import run_bass_kernel_spmd
from concourse.masks import make_identity

F32 = mybir.dt.float32
BF16 = mybir.dt.bfloat16
F8 = mybir.dt.float8e4
AF = mybir.ActivationFunctionType
DR = mybir.MatmulPerfMode.DoubleRow

B, T, C, U, P = 8, 2048, 512, 256, 128
TC = T // P    # 16 row tiles
CCH = C // P   # 4 c-chunks
UCH = U // P   # 2 u-chunks
TBLK = 512     # t-block for attention
NTB = T // TBLK  # 4
VF = U + 16    # v free dim padded so the pair-dim stride is 16B-aligned
SCALE = 1.0 / float(np.sqrt(U))
EXPB = -2.0    # exp bias; cancels in row-sum normalization

_cache = {}


WBYT = 3 * CCH * U + 16   # W3 | bqk bytes


def _build_kernel(tc):
    nc = tc.nc
    # one dma_start moves ~100-160GB/s and transfers are FIFO per issuing
    # ring, so inputs are spread over the three rings (sync, scalar,
    # gpsimd-SWDGE) sized/ordered by consumption deadline
    Wblob = nc.dram_tensor("Wblob", [P, WBYT], F8, kind="ExternalInput").ap()
    xT8_0 = nc.dram_tensor("xT8_0", [P, CCH, TBLK], F8,
                           kind="ExternalInput").ap()
    xT8_1 = nc.dram_tensor("xT8_1", [P, CCH, TBLK], F8,
                           kind="ExternalInput").ap()
    xT8_23 = nc.dram_tensor("xT8_23", [P, 2, CCH, TBLK], F8,
                            kind="ExternalInput").ap()
    Wa8 = nc.dram_tensor("Wa8", [P, UCH, C], F8, kind="ExternalInput").ap()
    xpb = nc.dram_tensor("xpb", [P, TC, C], BF16, kind="ExternalInput").ap()
    out = nc.dram_tensor("out", [P, TC, C], BF16, kind="ExternalOutput").ap()

    consts = tc.alloc_tile_pool(name="consts", bufs=1)
    persist = tc.alloc_tile_pool(name="persist", bufs=1)

    # warmup operands are plain memsets so the PE can start ramping the
    # HAM power state ~1.5us before make_identity's iota chain finishes
    warm_lhs = consts.tile([P, P], BF16)
    nc.vector.memset(warm_lhs, 0.0)
    warm_rhs = consts.tile([P, TBLK], BF16)
    nc.vector.memset(warm_rhs, 0.0)

    identity = consts.tile([P, P], BF16)
    make_identity(nc, identity)

    # warm the ACT exp table early (one-time ~2.7us table load)
    dex = consts.tile([P, 1], F32)
    nc.vector.memset(dex, 0.0)
    expb = consts.tile([P, 1], F32)
    nc.vector.memset(expb, EXPB)
    dex2 = consts.tile([P, 1], F32)
    nc.scalar.activation(out=dex2, in_=dex, func=AF.Exp, bias=dex[:, 0:1],
                         scale=1.0)

    # persistent tensors
    Wblob_sb = persist.tile([P, WBYT], F8)
    xT8_sb = persist.tile([P, NTB, CCH, TBLK], F8)  # x^T (c on partitions)
    Wa_f8 = persist.tile([P, UCH, C], F8)
    xpb_sb = persist.tile([P, TC, C], BF16)   # residual x + bc, bf16
    qT_f8 = persist.tile([P, UCH, T], F8)     # q^T  (u on partitions)
    kT_f8 = persist.tile([P, UCH, T], F8)     # k^T
    v_sb = persist.tile([P, TC, VF], F8)      # v rows + ones col + pad
    aT_f8 = persist.tile([P, UCH, T], F8)     # a^T (normalized)
    nc.vector.memset(v_sb[:, :, U:VF], 0.0)
    nc.vector.memset(v_sb[:, :, U:U + 1], 1.0)

    # views into the weight blob
    W3_sb = Wblob_sb[:, 0:3 * CCH * U].rearrange(
        "p (w c u) -> p w c u", w=3, c=CCH)
    Wq_f8, Wk_f8, Wv_f8 = W3_sb[:, 0], W3_sb[:, 1], W3_sb[:, 2]
    bqk_sb = Wblob_sb[:, 3 * CCH * U:WBYT].bitcast(F32)  # [P, 4]
    bq_sb, bk_sb = bqk_sb[:, 0:2], bqk_sb[:, 2:4]

    def xT8v(tb):
        return xT8_sb[:, tb]

    # sync ring: weights+biases, then the later x^T slices
    nc.sync.dma_start(out=Wblob_sb, in_=Wblob)
    nc.sync.dma_start(out=xT8_sb[:, 1], in_=xT8_1)
    nc.sync.dma_start(out=xT8_sb[:, 2:4], in_=xT8_23)
    # scalar ring: first x^T slice (preamble-critical), output weights
    nc.scalar.dma_start(out=xT8_sb[:, 0], in_=xT8_0)
    nc.scalar.dma_start(out=Wa_f8, in_=Wa8)
    # gpsimd (SWDGE) ring: the residual, needed only from block 1 on
    nc.gpsimd.dma_start(out=xpb_sb[:, 0:TC // 2, :], in_=xpb[:, 0:TC // 2, :])
    nc.gpsimd.dma_start(out=xpb_sb[:, TC // 2:, :], in_=xpb[:, TC // 2:, :])

    # PSUM pools for the whole kernel: 4 (scores) + 2 (acc) + 2 (misc)
    spsum = tc.alloc_tile_pool(name="spsum", bufs=2, space="PSUM")
    apsum = tc.alloc_tile_pool(name="apsum", bufs=2, space="PSUM")
    p_pool = tc.alloc_tile_pool(name="p_pool", bufs=14)
    abf_pool = tc.alloc_tile_pool(name="abf_pool", bufs=8)
    rcp_pool = tc.alloc_tile_pool(name="rcp_pool", bufs=3)
    y_pool = tc.alloc_tile_pool(name="y_pool", bufs=2)

    # PE warmup during the DMA stream (no data deps): bridges the wait
    # for the first input DMA and ramps the HAM power state (~4us of
    # continuous PE activity needed for full clock)
    wtile = apsum.tile([P, TBLK], F32, tag="misc", name="warmup")
    for i in range(8):
        nc.tensor.matmul(wtile, lhsT=warm_lhs, rhs=warm_rhs,
                         start=(i == 0), stop=(i == 7))

    def proj_chunk(W_f8, bias_sb, dst, uc, tb, eng_act):
        """q/k projection for one u-chunk x one t-block slice."""
        wps = apsum.tile([P, TBLK], F32, tag="misc", name="wps")
        for i in range(2):
            nc.tensor.matmul(
                wps,
                lhsT=W_f8[:, 2 * i:2 * i + 2, ts(uc, P)],
                rhs=xT8v(tb)[:, 2 * i:2 * i + 2, :],
                start=(i == 0), stop=(i == 1), perf_mode=DR,
            )
        if eng_act:
            nc.scalar.activation(
                out=dst[:, uc, ds(tb * TBLK, TBLK)], in_=wps,
                func=AF.Identity, bias=bias_sb[:, uc:uc + 1], scale=1.0,
            )
        else:
            nc.vector.tensor_scalar(
                out=dst[:, uc, ds(tb * TBLK, TBLK)], in0=wps,
                scalar1=bias_sb[:, uc:uc + 1], scalar2=None,
                op0=mybir.AluOpType.add,
            )

    def v_chunk(j, eng_act):
        """v projection for row tiles 2j, 2j+1 (one PSUM bank)."""
        tb, j0 = (2 * j) // 4, (2 * j) % 4
        vps = apsum.tile([P, 2, U], F32, tag="misc", name="vps")
        for jj in range(2):
            for i in range(2):
                nc.tensor.matmul(
                    vps[:, jj, :],
                    lhsT=xT8v(tb)[:, 2 * i:2 * i + 2,
                                  ds((j0 + jj) * P, P)],
                    rhs=Wv_f8[:, 2 * i:2 * i + 2, :],
                    start=(jj == 0 and i == 0),
                    stop=(jj == 1 and i == 1), perf_mode=DR,
                )
        t0 = 2 * j
        if eng_act:
            nc.scalar.copy(out=v_sb[:, t0:t0 + 2, 0:U], in_=vps)
        else:
            nc.vector.tensor_copy(out=v_sb[:, t0:t0 + 2, 0:U], in_=vps)

    # preamble: q^T block 0, k^T slice 0, v tiles 0-3 (needs xT8 slice 0).
    # Scalar/Vector alternate here (exp isn't running yet).
    for uc in range(UCH):
        proj_chunk(Wq_f8, bq_sb, qT_f8, uc, 0, uc == 0)
    for uc in range(UCH):
        proj_chunk(Wk_f8, bk_sb, kT_f8, uc, 0, True)
    v_chunk(0, True)
    v_chunk(1, False)

    def norm_cast(apss, abfs, tsl, eng_act=False):
        """rcp of row sum, then a_bf = aps * rcp (normalized), fp32->bf16."""
        aps = apss[tsl]
        rcp = rcp_pool.tile([P, 1], F32, tag="rcp")
        nc.vector.reciprocal(rcp, aps[:, U:U + 1])
        a_bf = abf_pool.tile([P, U], BF16, tag="abf")
        if eng_act:
            nc.scalar.activation(out=a_bf, in_=aps[:, 0:U], func=AF.Copy,
                                 scale=rcp[:, 0:1])
        else:
            nc.vector.tensor_scalar(
                out=a_bf, in0=aps[:, 0:U], scalar1=rcp, scalar2=None,
                op0=mybir.AluOpType.mult,
            )
        abfs[tsl] = a_bf

    def deferred_work(tb, abfs, single_dma=False):
        """Transposes of a (tb), then output projection + residual (tb).
        Returns (tchunks, fchunks); y tiles pair up for one DMA per two
        row tiles unless single_dma (used to shorten the final tail)."""
        tchunks, fchunks = [], []
        y2box = [None]
        for tsl in range(NTB):
            def tchunk(tsl=tsl, tb=tb, abfs=abfs):
                row0 = tb * TBLK + tsl * P
                atps = apsum.tile([P, UCH, P], F32, tag="misc", name="atps")
                for uc in range(UCH):
                    nc.tensor.matmul(
                        atps[:, uc, :], lhsT=abfs[tsl][:, ts(uc, P)],
                        rhs=identity,
                        start=(uc == 0), stop=(uc == UCH - 1),
                    )
                nc.vector.tensor_copy(out=aT_f8[:, :, ds(row0, P)], in_=atps)
            tchunks.append(tchunk)
        for tsl in range(NTB):
            def fchunk(tsl=tsl, tb=tb, y2box=y2box):
                row0 = tb * TBLK + tsl * P
                yps = apsum.tile([P, TBLK], F32, tag="misc", name="yps")
                nc.tensor.matmul(
                    yps, lhsT=aT_f8[:, :, ds(row0, P)], rhs=Wa_f8[:, :, :],
                    start=True, stop=True, perf_mode=DR,
                )
                if single_dma:
                    y1 = y_pool.tile([P, C], BF16, tag="ysb", name="y1")
                    nc.vector.tensor_add(
                        out=y1, in0=yps, in1=xpb_sb[:, tb * NTB + tsl, :]
                    )
                    # spread the final DMAs over rings that idle at the
                    # end so issue+transfer of the last tiles overlap
                    eng = (nc.gpsimd, nc.gpsimd, nc.sync, nc.scalar)[tsl]
                    eng.dma_start(out=out[:, tb * NTB + tsl, :], in_=y1)
                    return
                if tsl % 2 == 0:
                    y2box[0] = y_pool.tile([P, 2, C], BF16, tag="ysb",
                                           name="y2")
                y2 = y2box[0]
                nc.vector.tensor_add(
                    out=y2[:, tsl % 2, :], in0=yps,
                    in1=xpb_sb[:, tb * NTB + tsl, :]
                )
                if tsl % 2 == 1:
                    t0 = tb * NTB + tsl - 1
                    nc.gpsimd.dma_start(out=out[:, t0:t0 + 2, :], in_=y2)
            fchunks.append(fchunk)
        return tchunks, fchunks

    def emit_scp(tb, scp, pts):
        sps = spsum.tile([P, 2, TBLK], F32, tag="sps", name="sps")
        for j in range(2):
            nc.tensor.matmul(
                sps[:, j, :],
                lhsT=kT_f8[:, :, ts(2 * scp + j, P)],
                rhs=qT_f8[:, :, ds(tb * TBLK, TBLK)],
                start=True, stop=True, perf_mode=DR,
            )
        pt = p_pool.tile([P, 2, TBLK], F8, tag="pt", name="pt")
        nc.scalar.activation(out=pt, in_=sps, func=AF.Exp,
                             bias=expb[:, 0:1], scale=SCALE)
        pts.append(pt)

    # block 0's todo: remaining k^T slices / v tiles / q^T block 1, in an
    # order where each k^T slice lands before the scp that consumes it.
    # All on VectorE: ScalarE is saturated with exp during the streak.
    todo0 = []
    for tb in range(1, NTB):
        for uc in range(UCH):
            def kchunk(uc=uc, tb=tb):
                proj_chunk(Wk_f8, bk_sb, kT_f8, uc, tb, False)
            todo0.append(kchunk)
        for h in range(2):
            def vchunk(h=h, tb=tb):
                v_chunk(2 * tb + h, False)
            todo0.append(vchunk)

    deferred = []  # (streak chunks, sweep-B held-back chunks)
    heldB = []
    nextpts = []
    for tb in range(NTB):
        pts = nextpts  # scp0/1 may have been hoisted into tb-1's tail
        nextpts = []
        abfs = [None] * NTB
        apss = [None] * NTB
        for tsl in (0, 1):
            apss[tsl] = apsum.tile([P, VF], F32, tag="acc", name="apsA")
        todo = todo0 if tb == 0 else list(deferred)
        btodo = list(heldB)
        # qT for block tb+1 joins the streak queue (must finish before the
        # hoisted score groups of block tb+1 are emitted)
        if tb + 1 < NTB:
            for uc in range(UCH):
                def qchunk(uc=uc, tb=tb):
                    proj_chunk(Wq_f8, bq_sb, qT_f8, uc, tb + 1, False)
                todo.append(qchunk)
        npv = [0]  # sweep-A pairs emitted so far

        def pva_upto(limit):
            while npv[0] < limit:
                j = npv[0]
                for tsl in (0, 1):
                    nc.tensor.matmul(
                        apss[tsl],
                        lhsT=pts[j][:, :, ts(tsl, P)],
                        rhs=v_sb[:, 2 * j:2 * j + 2, :],
                        start=(j == 0), stop=False, perf_mode=DR,
                    )
                npv[0] += 1

        # interleave prep/deferred chunks into the streak: two per score
        # group for block 0 (it absorbs all k/v projections), one per
        # group later so PE filler survives into the late-streak
        # stretches where the PE would otherwise wait on the exp
        pop_at, rate = (1, 2) if tb == 0 else (2, 1)
        for scp in range(len(pts), 8):
            emit_scp(tb, scp, pts)
            # PV sweep A (row tiles 0,1), one pair behind the exp
            pva_upto(scp)
            if scp >= pop_at:
                for _ in range(rate):
                    if todo:
                        todo.pop(0)()
        pva_upto(7)
        for tsl in (0, 1):
            nc.tensor.matmul(
                apss[tsl], lhsT=pts[7][:, :, ts(tsl, P)],
                rhs=v_sb[:, 14:16, :], start=False, stop=True, perf_mode=DR,
            )
        while todo:
            todo.pop(0)()
        if tb + 1 < NTB:
            # hoist the next block's first two score groups so their
            # exps run on ScalarE while sweep B occupies the PE
            emit_scp(tb + 1, 0, nextpts)
            emit_scp(tb + 1, 1, nextpts)
        norm_cast(apss, abfs, 0)
        norm_cast(apss, abfs, 1)
        # PV sweep B (row tiles 2,3) over the retained p tiles, padded
        # with held-back chunks so the PE stays busy while ScalarE drains
        # the boundary exp backlog: the previous block's last two output
        # projections plus THIS block's first two a-transposes (they only
        # need norms 0,1, done just above)
        last = tb == NTB - 1
        for tsl in (2, 3):
            apss[tsl] = apsum.tile([P, VF], F32, tag="acc", name="apsB")
        if not last:
            tch, fch = deferred_work(tb, abfs)
            btodo = btodo + [tch[0], tch[1]]
            for scp in range(8):
                for tsl in (2, 3):
                    nc.tensor.matmul(
                        apss[tsl],
                        lhsT=pts[scp][:, :, ts(tsl, P)],
                        rhs=v_sb[:, 2 * scp:2 * scp + 2, :],
                        start=(scp == 0), stop=(scp == 7), perf_mode=DR,
                    )
                if scp % 2 == 1 and btodo:
                    btodo.pop(0)()
            norm_cast(apss, abfs, 2, True)
            norm_cast(apss, abfs, 3, True)
            while btodo:
                btodo.pop(0)()
            deferred = [fch[0], fch[1], tch[2], tch[3]]
            heldB = [fch[2], fch[3]]
        else:
            # last block: tile-SERIAL sweep B so tile 2's norm/transpose/
            # output-proj/DMA chain overlaps tile 3's PV, and only tile
            # 3's short chain remains as the tail (single-tile DMAs)
            tch, fch = deferred_work(tb, abfs, single_dma=True)
            btodo = btodo + [tch[0], tch[1], fch[0], fch[1]]
            for tsl in (2, 3):
                for scp in range(8):
                    nc.tensor.matmul(
                        apss[tsl],
                        lhsT=pts[scp][:, :, ts(tsl, P)],
                        rhs=v_sb[:, 2 * scp:2 * scp + 2, :],
                        start=(scp == 0), stop=(scp == 7), perf_mode=DR,
                    )
                    if scp % 2 == 1 and btodo:
                        btodo.pop(0)()
                if tsl == 2:
                    norm_cast(apss, abfs, 2, True)
                    btodo = btodo + [tch[2], fch[2]]
            norm_cast(apss, abfs, 3, True)
            while btodo:
                btodo.pop(0)()
            tch[3]()
            fch[3]()

    for pool in (y_pool, rcp_pool, abf_pool, p_pool,
                 apsum, spsum, persist, consts):
        pool.release()


def _get_nc():
    if "nc" not in _cache:
        nc = bacc.Bacc("TRN2", target_bir_lowering=False, debug=False)
        with tile.TileContext(nc) as tc:
            _build_kernel(tc)
        nc.compile()
        _cache["nc"] = nc
    return _cache["nc"]


def _w8(w, chunks):
    """fp32 [K, N] -> fp8e4m3 [P, K//P, N] with K-chunk layout for lhsT."""
    f8 = w.reshape(chunks, P, -1).transpose(1, 0, 2)
    return np.ascontiguousarray(f8.astype(ml_dtypes.float8_e4m3))


def _host_inputs(inputs):
    f32 = np.float32
    f8 = ml_dtypes.float8_e4m3
    Wa = np.ascontiguousarray(np.asarray(inputs["Wa"], dtype=f32))
    bc = np.asarray(inputs["bv"], dtype=f32) @ Wa + np.asarray(
        inputs["ba"], dtype=f32
    )
    W3 = np.stack([
        _w8(np.asarray(inputs["Wq"], dtype=f32), CCH),
        _w8(np.asarray(inputs["Wk"], dtype=f32), CCH),
        _w8(np.asarray(inputs["Wv"], dtype=f32), CCH),
    ], axis=1).reshape(P, -1)  # [P, 3*CCH*U]
    bqk = np.stack([
        np.asarray(inputs["bq"], dtype=f32).reshape(UCH, P).T,
        np.asarray(inputs["bk"], dtype=f32).reshape(UCH, P).T,
    ], axis=1)  # [P, 2, UCH]
    bqk_bytes = np.ascontiguousarray(bqk).view(np.uint8).reshape(P, 16)
    Wa_b = _w8(Wa, UCH).reshape(P, -1)
    Wblob = np.ascontiguousarray(
        np.concatenate([W3, bqk_bytes.view(f8)], axis=1))
    Wa8 = np.ascontiguousarray(Wa_b.reshape(P, UCH, C))
    xs = np.asarray(inputs["x"], dtype=f32)
    maps = []
    for b in range(B):
        # xT8[p, tb, cc, t'] = x[b][tb*TBLK + t', cc*P + p]  in fp8
        xt = xs[b].T.astype(f8)  # [C, T]
        xt = xt.reshape(CCH, P, NTB, TBLK).transpose(1, 2, 0, 3)
        # xpb[p, tt, c] = x[tt*P + p, c] + bc[c]  in bf16
        xpb = (xs[b] + bc).astype(ml_dtypes.bfloat16)
        xpb = xpb.reshape(TC, P, C).transpose(1, 0, 2)
        maps.append({
            "Wblob": Wblob,
            "Wa8": Wa8,
            "xT8_0": np.ascontiguousarray(xt[:, 0]),
            "xT8_1": np.ascontiguousarray(xt[:, 1]),
            "xT8_23": np.ascontiguousarray(xt[:, 2:4]),
            "xpb": np.ascontiguousarray(xpb),
        })
    return maps


def _unshard_out(o):
    """[P, TC, C] bf16 -> [T, C] f32."""
    return np.asarray(o).transpose(1, 0, 2).reshape(T, C).astype(np.float32)


def kernel(**inputs):
    nc = _get_nc()
    in_maps = _host_inputs(inputs)
    res = run_bass_kernel_spmd(nc, in_maps, core_ids=list(range(B)))
    return np.stack(
        [_unshard_out(res.results[b]["out"]) for b in range(B)], axis=0
    )


# revision 27
# speedup vs baseline: 1.0257x; 1.0228x over previous
"""Self-attention kernel for TRN2, data-parallel over batch (8 cores), fp8.

Per core (one batch element x[2048, 512]):
  - x^T is prepared on the HOST in fp8 ([P, NTB, CCH, TBLK] layout), so no
    on-chip transposes are needed; the residual x (+ folded bias bc) is
    shipped bf16 and the output is written bf16 (host casts back to fp32).
  - q/k/v projections and all attention matmuls run fp8 with
    perf_mode=DoubleRow (contraction pairs of 128-chunks -> ~2x TensorE).
  - scores computed TRANSPOSED (sT[s,t]) so the exp output feeds PV
    directly; exp = e^{score/16 - 2} (bias cancels in normalization),
    fused over two PSUM banks per activation.
  - PV runs lag-1 behind the exp; row sums come free via a ones-column
    in v; the reciprocal is folded into the bf16 cast of a.
  - block 0's score streak absorbs the k/v projection chunks the same way
    later blocks absorb the previous block's transposes/output projection;
    two output-projection chunks are held back into each sweep B so the
    PE has work while ScalarE drains the block-boundary exp backlog.
  - DMA issue cost (~0.6us per dma_start on a sequencer) is split across
    three queues: inputs on Sync + Scalar, outputs on GpSimd.
  - biases: bq/bk exact via per-partition add; bv/ba folded on the HOST
    into bc = Wa^T bv + ba, added into the bf16 residual (exact:
    attention rows sum to 1).

Matmul inputs fp8e4, PSUM accumulation fp32, softmax/normalize fp32,
residual + output bf16.
"""

import ml_dtypes
import numpy as np

import concourse.bass as bass
import concourse.mybir as mybir
import concourse.tile as tile
from concourse import bacc
from concourse.bass import ds, ts
from concourse.bass_utils import run_bass_kernel_spmd
from concourse.masks import make_identity

F32 = mybir.dt.float32
BF16 = mybir.dt.bfloat16
F8 = mybir.dt.float8e4
AF = mybir.ActivationFunctionType
DR = mybir.MatmulPerfMode.DoubleRow

B, T, C, U, P = 8, 2048, 512, 256, 128
TC = T // P    # 16 row tiles
CCH = C // P   # 4 c-chunks
UCH = U // P   # 2 u-chunks
TBLK = 512     # t-block for attention
NTB = T // TBLK  # 4
VF = U + 16    # v free dim padded so the pair-dim stride is 16B-aligned
SCALE = 1.0 / float(np.sqrt(U))
EXPB = -2.0    # exp bias; cancels in row-sum normalization

_cache = {}


WBYT = 3 * CCH * U + 16   # W3 | bqk bytes


def _build_kernel(tc):
    nc = tc.nc
    # one dma_start moves ~100-160GB/s and transfers are FIFO per issuing
    # ring, so inputs are spread over the three rings (sync, scalar,
    # gpsimd-SWDGE) sized/ordered by consumption deadline
    Wblob = nc.dram_tensor("Wblob", [P, WBYT], F8, kind="ExternalInput").ap()
    xT8_0 = nc.dram_tensor("xT8_0", [P, CCH, TBLK], F8,
                           kind="ExternalInput").ap()
    xT8_1 = nc.dram_tensor("xT8_1", [P, CCH, TBLK], F8,
                           kind="ExternalInput").ap()
    xT8_23 = nc.dram_tensor("xT8_23", [P, 2, CCH, TBLK], F8,
                            kind="ExternalInput").ap()
    Wa8 = nc.dram_tensor("Wa8", [P, UCH, C], F8, kind="ExternalInput").ap()
    xpb = nc.dram_tensor("xpb", [P, TC, C], BF16, kind="ExternalInput").ap()
    out = nc.dram_tensor("out", [P, TC, C], BF16, kind="ExternalOutput").ap()

    consts = tc.alloc_tile_pool(name="consts", bufs=1)
    persist = tc.alloc_tile_pool(name="persist", bufs=1)

    # warmup operands are plain memsets so the PE can start ramping the
    # HAM power state ~1.5us before make_identity's iota chain finishes
    warm_lhs = consts.tile([P, P], BF16)
    nc.vector.memset(warm_lhs, 0.0)
    warm_rhs = consts.tile([P, TBLK], BF16)
    nc.vector.memset(warm_rhs, 0.0)

    identity = consts.tile([P, P], BF16)
    make_identity(nc, identity)

    # warm the ACT exp table early (one-time ~2.7us table load)
    dex = consts.tile([P, 1], F32)
    nc.vector.memset(dex, 0.0)
    expb = consts.tile([P, 1], F32)
    nc.vector.memset(expb, EXPB)
    dex2 = consts.tile([P, 1], F32)
    nc.scalar.activation(out=dex2, in_=dex, func=AF.Exp, bias=dex[:, 0:1],
                         scale=1.0)

    # persistent tensors
    Wblob_sb = persist.tile([P, WBYT], F8)
    xT8_sb = persist.tile([P, NTB, CCH, TBLK], F8)  # x^T (c on partitions)
    Wa_f8 = persist.tile([P, UCH, C], F8)
    xpb_sb = persist.tile([P, TC, C], BF16)   # residual x + bc, bf16
    qT_f8 = persist.tile([P, UCH, T], F8)     # q^T  (u on partitions)
    kT_f8 = persist.tile([P, UCH, T], F8)     # k^T
    v_sb = persist.tile([P, TC, VF], F8)      # v rows + ones col + pad
    aT_f8 = persist.tile([P, UCH, T], F8)     # a^T (normalized)
    nc.vector.memset(v_sb[:, :, U:VF], 0.0)
    nc.vector.memset(v_sb[:, :, U:U + 1], 1.0)

    # views into the weight blob
    W3_sb = Wblob_sb[:, 0:3 * CCH * U].rearrange(
        "p (w c u) -> p w c u", w=3, c=CCH)
    Wq_f8, Wk_f8, Wv_f8 = W3_sb[:, 0], W3_sb[:, 1], W3_sb[:, 2]
    bqk_sb = Wblob_sb[:, 3 * CCH * U:WBYT].bitcast(F32)  # [P, 4]
    bq_sb, bk_sb = bqk_sb[:, 0:2], bqk_sb[:, 2:4]

    def xT8v(tb):
        return xT8_sb[:, tb]

    # sync ring: weights+biases, then the later x^T slices
    nc.sync.dma_start(out=Wblob_sb, in_=Wblob)
    nc.sync.dma_start(out=xT8_sb[:, 1], in_=xT8_1)
    nc.sync.dma_start(out=xT8_sb[:, 2:4], in_=xT8_23)
    # scalar ring: first x^T slice (preamble-critical), output weights
    nc.scalar.dma_start(out=xT8_sb[:, 0], in_=xT8_0)
    nc.scalar.dma_start(out=Wa_f8, in_=Wa8)
    # gpsimd (SWDGE) ring: the residual, needed only from block 1 on
    nc.gpsimd.dma_start(out=xpb_sb[:, 0:TC // 2, :], in_=xpb[:, 0:TC // 2, :])
    nc.gpsimd.dma_start(out=xpb_sb[:, TC // 2:, :], in_=xpb[:, TC // 2:, :])

    # PSUM pools for the whole kernel: 4 (scores) + 2 (acc) + 2 (misc)
    spsum = tc.alloc_tile_pool(name="spsum", bufs=2, space="PSUM")
    apsum = tc.alloc_tile_pool(name="apsum", bufs=2, space="PSUM")
    p_pool = tc.alloc_tile_pool(name="p_pool", bufs=14)
    abf_pool = tc.alloc_tile_pool(name="abf_pool", bufs=8)
    rcp_pool = tc.alloc_tile_pool(name="rcp_pool", bufs=3)
    y_pool = tc.alloc_tile_pool(name="y_pool", bufs=2)

    # PE warmup during the DMA stream (no data deps): bridges the wait
    # for the first input DMA and ramps the HAM power state (~4us of
    # continuous PE activity needed for full clock)
    wtile = apsum.tile([P, TBLK], F32, tag="misc", name="warmup")
    for i in range(8):
        nc.tensor.matmul(wtile, lhsT=warm_lhs, rhs=warm_rhs,
                         start=(i == 0), stop=(i == 7))

    def proj_chunk(W_f8, bias_sb, dst, uc, tb, eng_act):
        """q/k projection for one u-chunk x one t-block slice."""
        wps = apsum.tile([P, TBLK], F32, tag="misc", name="wps")
        for i in range(2):
            nc.tensor.matmul(
                wps,
                lhsT=W_f8[:, 2 * i:2 * i + 2, ts(uc, P)],
                rhs=xT8v(tb)[:, 2 * i:2 * i + 2, :],
                start=(i == 0), stop=(i == 1), perf_mode=DR,
            )
        if eng_act:
            nc.scalar.activation(
                out=dst[:, uc, ds(tb * TBLK, TBLK)], in_=wps,
                func=AF.Identity, bias=bias_sb[:, uc:uc + 1], scale=1.0,
            )
        else:
            nc.vector.tensor_scalar(
                out=dst[:, uc, ds(tb * TBLK, TBLK)], in0=wps,
                scalar1=bias_sb[:, uc:uc + 1], scalar2=None,
                op0=mybir.AluOpType.add,
            )

    def v_chunk(j, eng_act):
        """v projection for row tiles 2j, 2j+1 (one PSUM bank)."""
        tb, j0 = (2 * j) // 4, (2 * j) % 4
        vps = apsum.tile([P, 2, U], F32, tag="misc", name="vps")
        for jj in range(2):
            for i in range(2):
                nc.tensor.matmul(
                    vps[:, jj, :],
                    lhsT=xT8v(tb)[:, 2 * i:2 * i + 2,
                                  ds((j0 + jj) * P, P)],
                    rhs=Wv_f8[:, 2 * i:2 * i + 2, :],
                    start=(jj == 0 and i == 0),
                    stop=(jj == 1 and i == 1), perf_mode=DR,
                )
        t0 = 2 * j
        if eng_act:
            nc.scalar.copy(out=v_sb[:, t0:t0 + 2, 0:U], in_=vps)
        else:
            nc.vector.tensor_copy(out=v_sb[:, t0:t0 + 2, 0:U], in_=vps)

    # preamble: q^T block 0, k^T slice 0, v tiles 0-3 (needs xT8 slice 0).
    # Scalar/Vector alternate here (exp isn't running yet).
    for uc in range(UCH):
        proj_chunk(Wq_f8, bq_sb, qT_f8, uc, 0, uc == 0)
    for uc in range(UCH):
        proj_chunk(Wk_f8, bk_sb, kT_f8, uc, 0, True)
    v_chunk(0, True)
    v_chunk(1, False)

    def norm_cast(apss, abfs, tsl, eng_act=False):
        """rcp of row sum, then a_bf = aps * rcp (normalized), fp32->bf16."""
        aps = apss[tsl]
        rcp = rcp_pool.tile([P, 1], F32, tag="rcp")
        nc.vector.reciprocal(rcp, aps[:, U:U + 1])
        a_bf = abf_pool.tile([P, U], BF16, tag="abf")
        if eng_act:
            nc.scalar.activation(out=a_bf, in_=aps[:, 0:U], func=AF.Copy,
                                 scale=rcp[:, 0:1])
        else:
            nc.vector.tensor_scalar(
                out=a_bf, in0=aps[:, 0:U], scalar1=rcp, scalar2=None,
                op0=mybir.AluOpType.mult,
            )
        abfs[tsl] = a_bf

    def deferred_work(tb, abfs, single_dma=False):
        """Transposes of a (tb), then output projection + residual (tb).
        Returns (tchunks, fchunks); y tiles pair up for one DMA per two
        row tiles unless single_dma (used to shorten the final tail)."""
        tchunks, fchunks = [], []
        y2box = [None]
        for tsl in range(NTB):
            def tchunk(tsl=tsl, tb=tb, abfs=abfs):
                row0 = tb * TBLK + tsl * P
                atps = apsum.tile([P, UCH, P], F32, tag="misc", name="atps")
                for uc in range(UCH):
                    nc.tensor.matmul(
                        atps[:, uc, :], lhsT=abfs[tsl][:, ts(uc, P)],
                        rhs=identity,
                        start=(uc == 0), stop=(uc == UCH - 1),
                    )
                nc.vector.tensor_copy(out=aT_f8[:, :, ds(row0, P)], in_=atps)
            tchunks.append(tchunk)
        for tsl in range(NTB):
            def fchunk(tsl=tsl, tb=tb, y2box=y2box):
                row0 = tb * TBLK + tsl * P
                yps = apsum.tile([P, TBLK], F32, tag="misc", name="yps")
                nc.tensor.matmul(
                    yps, lhsT=aT_f8[:, :, ds(row0, P)], rhs=Wa_f8[:, :, :],
                    start=True, stop=True, perf_mode=DR,
                )
                if single_dma:
                    y1 = y_pool.tile([P, C], BF16, tag="ysb", name="y1")
                    nc.vector.tensor_add(
                        out=y1, in0=yps, in1=xpb_sb[:, tb * NTB + tsl, :]
                    )
                    # spread the final DMAs over rings that idle at the
                    # end so issue+transfer of the last tiles overlap
                    eng = (nc.gpsimd, nc.gpsimd, nc.sync, nc.scalar)[tsl]
                    eng.dma_start(out=out[:, tb * NTB + tsl, :], in_=y1)
                    return
                if tsl % 2 == 0:
                    y2box[0] = y_pool.tile([P, 2, C], BF16, tag="ysb",
                                           name="y2")
                y2 = y2box[0]
                nc.vector.tensor_add(
                    out=y2[:, tsl % 2, :], in0=yps,
                    in1=xpb_sb[:, tb * NTB + tsl, :]
                )
                if tsl % 2 == 1:
                    t0 = tb * NTB + tsl - 1
                    nc.gpsimd.dma_start(out=out[:, t0:t0 + 2, :], in_=y2)
            fchunks.append(fchunk)
        return tchunks, fchunks

    def emit_scp(tb, scp, pts):
        sps = spsum.tile([P, 2, TBLK], F32, tag="sps", name="sps")
        for j in range(2):
            nc.tensor.matmul(
                sps[:, j, :],
                lhsT=kT_f8[:, :, ts(2 * scp + j, P)],
                rhs=qT_f8[:, :, ds(tb * TBLK, TBLK)],
                start=True, stop=True, perf_mode=DR,
            )
        pt = p_pool.tile([P, 2, TBLK], F8, tag="pt", name="pt")
        nc.scalar.activation(out=pt, in_=sps, func=AF.Exp,
                             bias=expb[:, 0:1], scale=SCALE)
        pts.append(pt)

    # block 0's todo: remaining k^T slices / v tiles / q^T block 1, in an
    # order where each k^T slice lands before the scp that consumes it.
    # All on VectorE: ScalarE is saturated with exp during the streak.
    todo0 = []
    for tb in range(1, NTB):
        for uc in range(UCH):
            def kchunk(uc=uc, tb=tb):
                proj_chunk(Wk_f8, bk_sb, kT_f8, uc, tb, False)
            todo0.append(kchunk)
        for h in range(2):
            def vchunk(h=h, tb=tb):
                v_chunk(2 * tb + h, False)
            todo0.append(vchunk)

    deferred = []  # (streak chunks, sweep-B held-back chunks)
    heldB = []
    nextpts = []
    for tb in range(NTB):
        pts = nextpts  # scp0/1 may have been hoisted into tb-1's tail
        nextpts = []
        abfs = [None] * NTB
        apss = [None] * NTB
        for tsl in (0, 1):
            apss[tsl] = apsum.tile([P, VF], F32, tag="acc", name="apsA")
        todo = todo0 if tb == 0 else list(deferred)
        btodo = list(heldB)
        # qT for block tb+1 joins the streak queue (must finish before the
        # hoisted score groups of block tb+1 are emitted)
        if tb + 1 < NTB:
            for uc in range(UCH):
                def qchunk(uc=uc, tb=tb):
                    proj_chunk(Wq_f8, bq_sb, qT_f8, uc, tb + 1, False)
                todo.append(qchunk)
        npv = [0]  # sweep-A pairs emitted so far

        def pva_upto(limit):
            while npv[0] < limit:
                j = npv[0]
                for tsl in (0, 1):
                    nc.tensor.matmul(
                        apss[tsl],
                        lhsT=pts[j][:, :, ts(tsl, P)],
                        rhs=v_sb[:, 2 * j:2 * j + 2, :],
                        start=(j == 0), stop=False, perf_mode=DR,
                    )
                npv[0] += 1

        # interleave prep/deferred chunks into the streak: two per score
        # group for block 0 (it absorbs all k/v projections), one per
        # group later so PE filler survives into the late-streak
        # stretches where the PE would otherwise wait on the exp
        pop_at, rate = (1, 2) if tb == 0 else (2, 1)
        for scp in range(len(pts), 8):
            emit_scp(tb, scp, pts)
            # PV sweep A (row tiles 0,1), two pairs behind the exp so
            # the PE never catches up to ScalarE's exp cadence
            pva_upto(max(0, scp - 1))
            if scp >= pop_at:
                for _ in range(rate):
                    if todo:
                        todo.pop(0)()
        pva_upto(7)
        for tsl in (0, 1):
            nc.tensor.matmul(
                apss[tsl], lhsT=pts[7][:, :, ts(tsl, P)],
                rhs=v_sb[:, 14:16, :], start=False, stop=True, perf_mode=DR,
            )
        while todo:
            todo.pop(0)()
        if tb + 1 < NTB:
            # hoist the next block's first two score groups so their
            # exps run on ScalarE while sweep B occupies the PE
            emit_scp(tb + 1, 0, nextpts)
            emit_scp(tb + 1, 1, nextpts)
        norm_cast(apss, abfs, 0)
        norm_cast(apss, abfs, 1)
        # PV sweep B (row tiles 2,3) over the retained p tiles, padded
        # with held-back chunks so the PE stays busy while ScalarE drains
        # the boundary exp backlog: the previous block's last two output
        # projections plus THIS block's first two a-transposes (they only
        # need norms 0,1, done just above)
        last = tb == NTB - 1
        for tsl in (2, 3):
            apss[tsl] = apsum.tile([P, VF], F32, tag="acc", name="apsB")
        if not last:
            tch, fch = deferred_work(tb, abfs)
            btodo = btodo + [tch[0], tch[1]]
            for scp in range(8):
                for tsl in (2, 3):
                    nc.tensor.matmul(
                        apss[tsl],
                        lhsT=pts[scp][:, :, ts(tsl, P)],
                        rhs=v_sb[:, 2 * scp:2 * scp + 2, :],
                        start=(scp == 0), stop=(scp == 7), perf_mode=DR,
                    )
                if scp % 2 == 1 and btodo:
                    btodo.pop(0)()
            norm_cast(apss, abfs, 2, True)
            norm_cast(apss, abfs, 3, True)
            while btodo:
                btodo.pop(0)()
            deferred = [fch[0], fch[1], tch[2], tch[3]]
            heldB = [fch[2], fch[3]]
        else:
            # last block: tile-SERIAL sweep B so tile 2's norm/transpose/
            # output-proj/DMA chain overlaps tile 3's PV, and only tile
            # 3's short chain remains as the tail (single-tile DMAs)
            tch, fch = deferred_work(tb, abfs, single_dma=True)
            btodo = btodo + [tch[0], tch[1], fch[0], fch[1]]
            for tsl in (2, 3):
                for scp in range(8):
                    nc.tensor.matmul(
                        apss[tsl],
                        lhsT=pts[scp][:, :, ts(tsl, P)],
                        rhs=v_sb[:, 2 * scp:2 * scp + 2, :],
                        start=(scp == 0), stop=(scp == 7), perf_mode=DR,
                    )
                    if scp % 2 == 1 and btodo:
                        btodo.pop(0)()
                if tsl == 2:
                    norm_cast(apss, abfs, 2, True)
                    btodo = btodo + [tch[2], fch[2]]
            norm_cast(apss, abfs, 3, True)
            while btodo:
                btodo.pop(0)()
            tch[3]()
            fch[3]()

    for pool in (y_pool, rcp_pool, abf_pool, p_pool,
                 apsum, spsum, persist, consts):
        pool.release()


def _get_nc():
    if "nc" not in _cache:
        nc = bacc.Bacc("TRN2", target_bir_lowering=False, debug=False)
        with tile.TileContext(nc) as tc:
            _build_kernel(tc)
        nc.compile()
        _cache["nc"] = nc
    return _cache["nc"]


def _w8(w, chunks):
    """fp32 [K, N] -> fp8e4m3 [P, K//P, N] with K-chunk layout for lhsT."""
    f8 = w.reshape(chunks, P, -1).transpose(1, 0, 2)
    return np.ascontiguousarray(f8.astype(ml_dtypes.float8_e4m3))


def _host_inputs(inputs):
    f32 = np.float32
    f8 = ml_dtypes.float8_e4m3
    Wa = np.ascontiguousarray(np.asarray(inputs["Wa"], dtype=f32))
    bc = np.asarray(inputs["bv"], dtype=f32) @ Wa + np.asarray(
        inputs["ba"], dtype=f32
    )
    W3 = np.stack([
        _w8(np.asarray(inputs["Wq"], dtype=f32), CCH),
        _w8(np.asarray(inputs["Wk"], dtype=f32), CCH),
        _w8(np.asarray(inputs["Wv"], dtype=f32), CCH),
    ], axis=1).reshape(P, -1)  # [P, 3*CCH*U]
    bqk = np.stack([
        np.asarray(inputs["bq"], dtype=f32).reshape(UCH, P).T,
        np.asarray(inputs["bk"], dtype=f32).reshape(UCH, P).T,
    ], axis=1)  # [P, 2, UCH]
    bqk_bytes = np.ascontiguousarray(bqk).view(np.uint8).reshape(P, 16)
    Wa_b = _w8(Wa, UCH).reshape(P, -1)
    Wblob = np.ascontiguousarray(
        np.concatenate([W3, bqk_bytes.view(f8)], axis=1))
    Wa8 = np.ascontiguousarray(Wa_b.reshape(P, UCH, C))
    xs = np.asarray(inputs["x"], dtype=f32)
    maps = []
    for b in range(B):
        # xT8[p, tb, cc, t'] = x[b][tb*TBLK + t', cc*P + p]  in fp8
        xt = xs[b].T.astype(f8)  # [C, T]
        xt = xt.reshape(CCH, P, NTB, TBLK).transpose(1, 2, 0, 3)
        # xpb[p, tt, c] = x[tt*P + p, c] + bc[c]  in bf16
        xpb = (xs[b] + bc).astype(ml_dtypes.bfloat16)
        xpb = xpb.reshape(TC, P, C).transpose(1, 0, 2)
        maps.append({
            "Wblob": Wblob,
            "Wa8": Wa8,
            "xT8_0": np.ascontiguousarray(xt[:, 0]),
            "xT8_1": np.ascontiguousarray(xt[:, 1]),
            "xT8_23": np.ascontiguousarray(xt[:, 2:4]),
            "xpb": np.ascontiguousarray(xpb),
        })
    return maps


def _unshard_out(o):
    """[P, TC, C] bf16 -> [T, C] f32."""
    return np.asarray(o).transpose(1, 0, 2).reshape(T, C).astype(np.float32)


def kernel(**inputs):
    nc = _get_nc()
    in_maps = _host_inputs(inputs)
    res = run_bass_kernel_spmd(nc, in_maps, core_ids=list(range(B)))
    return np.stack(
        [_unshard_out(res.results[b]["out"]) for b in range(B)], axis=0
    )
